# revision 2
# baseline (speedup 1.0000x reference)
"""Trainium2 Bass kernel for nn_Coo2Cel (periodic pairwise displacement /
squared-distance / cutoff-mask over all N x N atom pairs).

Math (reference semantics, minimum-image with diagonal-dominant cell):
    frac = pos @ inv(cel)                       # host, tiny
    d    = frac[j] - frac[i]                    # per pair, per component
    w    = d - round(d)                         # minimum image (pbc all-true)
    vec  = w * L                                # diagonal cell: L = cel[c,c]
    sod  = (vx^2 + vy^2) + vz^2
    mask = (sod < rc^2) & (i != j)
    vec, sod masked to 0 outside mask.

Sharding: i-axis of the NxN pair map, 8 slabs of (B*N)/8 = 1024 query rows;
each core holds all N positions (tiny) and writes its output slab.

Device-side structure per core (SPMD, identical program, per-core data):
  - frac_j rows broadcast to all 128 partitions by a partition-stride-0 DMA.
  - w via the existing ADD_RANGE_WRAP custom-DVE op (in0=frac_j, s0=-frac_i):
    out = y + 1.0*((y < -0.5) - (y > 0.5)), y = frac_j - frac_i  == d-round(d)
    bit-exact vs round-nearest-even for |d| < 1.5.
  - squares on ScalarE (Square activation, scale=L fused), sums on GPSIMD,
    Sign/Relu chain on ScalarE for the {L,0} mask multiplier and uint8 mask,
    masked vec written with an interleaved (j,c) stride so the DRAM store is
    contiguous [i, j, 3] rows.
Engines are balanced so DVE/POOL/ACT all run near the HBM-write roofline.
"""

import numpy as np

RC = 6.0
RC2 = np.float32(RC * RC)
B, N = 2, 4096
NCORES = 8
ROWS = B * N // NCORES          # 1024 i-rows per core
ITILES = ROWS // 128            # 8
FD = 2048                       # j-chunk width
NJC = N // FD

_BUILT = {}


def _build_nc(L):
    """Trace + compile the SPMD program (same NEFF for all 8 cores)."""
    from contextlib import ExitStack
    import concourse.bacc as bacc
    import concourse.tile as tile
    import concourse.mybir as mybir
    from concourse.dve_ops import ADD_RANGE_WRAP

    f32 = mybir.dt.float32
    u8 = mybir.dt.uint8
    Act = mybir.ActivationFunctionType
    Alu = mybir.AluOpType

    nc = bacc.Bacc("TRN2", target_bir_lowering=False, debug=False,
                   num_devices=NCORES)

    fj_d = nc.dram_tensor("fj", [3, N], f32, kind="ExternalInput")
    nfi_d = nc.dram_tensor("nfi", [128, ITILES * 3], f32, kind="ExternalInput")
    vec_d = nc.dram_tensor("vec_s", [ROWS, N, 3], f32, kind="ExternalOutput")
    sod_d = nc.dram_tensor("sod_s", [ROWS, N], f32, kind="ExternalOutput")
    mask_d = nc.dram_tensor("mask_s", [ROWS, N], u8, kind="ExternalOutput")

    # const AP for the Sign bias (activation converts float bias -> const AP)
    cb = nc.alloc_sbuf_tensor("const-f32-negrc2", [128, 1], f32)
    nc.gpsimd.memset(cb.ap(), float(-RC2))
    nc.const_aps.aps[(f32, float(-RC2))] = cb.ap()
    nc.all_engine_barrier()

    with tile.TileContext(nc) as tc, ExitStack() as ctx:
        fj_pool = ctx.enter_context(tc.tile_pool(name="fjp", bufs=2))
        w_pool = ctx.enter_context(tc.tile_pool(name="wp", bufs=1))
        sq_pool = ctx.enter_context(tc.tile_pool(name="sqp", bufs=1))
        mid_pool = ctx.enter_context(tc.tile_pool(name="midp", bufs=1))
        out_pool = ctx.enter_context(tc.tile_pool(name="outp", bufs=2))
        nfi_pool = ctx.enter_context(tc.tile_pool(name="nfip", bufs=1))

        nfi_t = nfi_pool.tile([128, ITILES * 3], f32, tag="nfi")
        nc.sync.dma_start(nfi_t[:], nfi_d.ap())

        for jc in range(NJC):
            j0 = jc * FD
            fjt = []
            for c in range(3):
                t = fj_pool.tile([128, FD], f32, tag=f"fj{c}", name=f"fj{c}")
                nc.sync.dma_start(
                    t[:], fj_d.ap()[c:c + 1, j0:j0 + FD].broadcast_to([128, FD]))
                fjt.append(t)

            for it in range(ITILES):
                r0 = it * 128
                w = [w_pool.tile([128, FD], f32, tag=f"w{c}", name=f"w{c}")
                     for c in range(3)]
                for c in range(3):
                    sc = it * 3 + c
                    nc.vector._custom_dve(
                        ADD_RANGE_WRAP, out=w[c][:], in0=fjt[c][:],
                        s0=nfi_t[:, sc:sc + 1], s1=0.5, imm2=1.0)

                sq = [sq_pool.tile([128, FD], f32, tag=f"sq{c}", name=f"sq{c}")
                      for c in range(3)]
                for c in range(3):
                    nc.scalar.activation(sq[c][:], w[c][:], Act.Square,
                                         scale=float(L[c]))

                s01 = mid_pool.tile([128, FD], f32, tag="s01")
                nc.gpsimd.tensor_tensor(s01[:], sq[0][:], sq[1][:], Alu.add)
                sod = mid_pool.tile([128, FD], f32, tag="sod")
                nc.gpsimd.tensor_tensor(sod[:], s01[:], sq[2][:], Alu.add)

                sg = mid_pool.tile([128, FD], f32, tag="sg")
                nc.scalar.activation(sg[:], sod[:], Act.Sign, bias=float(-RC2))
                msc = mid_pool.tile([128, FD], f32, tag="msc")
                nc.scalar.activation(msc[:], sg[:], Act.Relu,
                                     scale=float(-L[0]))
                m8 = out_pool.tile([128, FD], u8, tag="m8")
                nc.scalar.activation(m8[:], sg[:], Act.Relu, scale=-1.0)

                sodm = out_pool.tile([128, FD], f32, tag="sodm")
                nc.vector.tensor_tensor(sodm[:], sod[:], msc[:], Alu.min)

                V = out_pool.tile([128, FD * 3], f32, tag="V")
                V3 = V[:].rearrange("p (j c) -> p j c", c=3)
                nc.vector.tensor_tensor(V3[:, :, 2], w[2][:], msc[:], Alu.mult)
                nc.vector.tensor_tensor(V3[:, :, 1], w[1][:], msc[:], Alu.mult)
                nc.gpsimd.tensor_tensor(V3[:, :, 0], w[0][:], msc[:], Alu.mult)

                nc.sync.dma_start(vec_d.ap()[r0:r0 + 128, j0:j0 + FD, :], V3)
                nc.sync.dma_start(sod_d.ap()[r0:r0 + 128, j0:j0 + FD],
                                  sodm[:])
                nc.scalar.dma_start(mask_d.ap()[r0:r0 + 128, j0:j0 + FD],
                                    m8[:])

    nc.compile()
    return nc


def _fallback(pos_xyz, cel_mat, pbc, ent):
    """Exact numpy replica of the reference for inputs outside the fast path
    (non-diagonal cell, partial pbc/ent, wide frac range)."""
    inv = np.linalg.inv(cel_mat.astype(np.float64)).astype(np.float32)
    frac = np.einsum('bni,bij->bnj', pos_xyz, inv).astype(np.float32)
    dfrac = frac[:, None, :, :] - frac[:, :, None, :]
    shift = np.round(dfrac) * pbc[:, None, None, :].astype(dfrac.dtype)
    dfrac = (dfrac - shift).astype(np.float32)
    vec = np.einsum('bijk,bkl->bijl', dfrac, cel_mat).astype(np.float32)
    sod = np.sum(vec * vec, axis=-1)
    n = pos_xyz.shape[1]
    pair_ok = (ent[:, :, None] & ent[:, None, :]
               & ~np.eye(n, dtype=bool)[None])
    mask = pair_ok & (sod < RC * RC)
    vec = np.where(mask[..., None], vec, 0.0).astype(np.float32)
    sod = np.where(mask, sod, 0.0).astype(np.float32)
    return vec, sod, mask


def kernel(pos_xyz, cel_mat, pbc, ent):
    from concourse.bass_utils import run_bass_kernel_spmd

    pos_xyz = np.asarray(pos_xyz, dtype=np.float32)
    cel_mat = np.asarray(cel_mat, dtype=np.float32)
    pbc = np.asarray(pbc, dtype=bool)
    ent = np.asarray(ent, dtype=bool)
    assert pos_xyz.shape == (B, N, 3)

    # host: frac exactly as the reference computes it (jax CPU)
    import jax
    import jax.numpy as jnp
    cpu = jax.devices("cpu")[0]
    with jax.default_device(cpu):
        inv = jnp.linalg.inv(jnp.asarray(cel_mat))
        frac = jnp.einsum('bni,bij->bnj', jnp.asarray(pos_xyz), inv)
        frac = np.asarray(jax.device_get(frac), dtype=np.float32)

    offdiag = cel_mat * (1.0 - np.eye(3, dtype=np.float32))
    L = np.diagonal(cel_mat, axis1=1, axis2=2)  # [B, 3]
    fast = (
        np.all(offdiag == 0.0)
        and np.all(L == L[0, 0])                 # one uniform L
        and np.all(pbc) and np.all(ent)
        and (frac.max() - frac.min()) < 1.45     # wrap-by-one-period valid
    )
    if not fast:
        return _fallback(pos_xyz, cel_mat, pbc, ent)

    Lc = L[0]  # [3], all equal here
    key = ("coo2cel", tuple(np.float64(Lc)))
    if key not in _BUILT:
        _BUILT[key] = _build_nc(Lc)
    nc = _BUILT[key]

    # per-core inputs: core k owns flat rows [k*ROWS, (k+1)*ROWS) of (b, i)
    in_maps = []
    for k in range(NCORES):
        b = (k * ROWS) // N
        i0 = (k * ROWS) % N
        fj = np.ascontiguousarray(frac[b].T)                   # [3, N]
        fslab = frac[b, i0:i0 + ROWS, :]                       # [ROWS, 3]
        nfi = np.ascontiguousarray(
            -fslab.reshape(ITILES, 128, 3).transpose(1, 0, 2)
            .reshape(128, ITILES * 3))
        in_maps.append({"fj": fj, "nfi": nfi})

    res = run_bass_kernel_spmd(nc, in_maps, core_ids=list(range(NCORES)))

    vec = np.empty((B, N, N, 3), dtype=np.float32)
    sod = np.empty((B, N, N), dtype=np.float32)
    mask = np.empty((B, N, N), dtype=np.uint8)
    for k in range(NCORES):
        b = (k * ROWS) // N
        i0 = (k * ROWS) % N
        r = res.results[k]
        vec[b, i0:i0 + ROWS] = r["vec_s"]
        sod[b, i0:i0 + ROWS] = r["sod_s"]
        mask[b, i0:i0 + ROWS] = r["mask_s"]

    idx = np.arange(N)
    mask[:, idx, idx] = 0          # i == j excluded (vec/sod already 0 there)
    return vec, sod, mask.view(np.bool_)
